# revision 26
# baseline (speedup 1.0000x reference)
"""Multi-head attention (B=2, N=2048, C=1024, H=16) on 8 Trainium2 NeuronCores.

Sharding: tensor-parallel over heads (2 heads/core) for qkv-proj + attention;
all-to-all of the attention output in 5 stages (1024,1024,1024,512,512 tokens,
smaller tail stages to shrink the serial end), then each core runs the output
projection over the full channel dim for its token slices.

Per-core layouts:
  x^T   32 tiles [128, 1024] via HW DMA-transpose, issued on BOTH hwdge
        queues (sync + scalar) so the first qkv matmul starts ~5us in
  q^T/k^T/v^T per 512-token tile [128, 512]: rows 0:63 head A, 64:127 head B
        (natural packing; S matmuls use K=64 with tile_position from base
        partition -- no zero-padding, one PSUM->SBUF copy per tile)
  S^T   [128 keys, 1024] psum groups (2 key-chunks x 512 queries), 2 bufs
        cycling heads; exp on ScalarE (bf16 out, scale folded)
  vni   [128, 130] = [v_A|1|v_B|1] per key-chunk (ones col -> softmax denom)
  out_u [65, 512] psum accumulated over all 16 key chunks; normalization
        reads psum directly: reciprocal_approx_fast + ones-row broadcast
        matmul + DVE mul -> outT[h] [64, 4096] bf16
  a2a   one staging DMA per (stage, head) via rearranged APs; one unstage
        DMA per stage; oproj accumulates 8 K-chunks + bias matmul

Emission order software-pipelines the engines: the exp-bound attention inner
loop is padded with "filler" closures (qkv of next batch, v transposes,
oproj of previous stage) popped between groups so the PE never idles (keeps
the tensor-engine p-state at full clock).
"""

import numpy as np
import ml_dtypes
from collections import deque
from contextlib import ExitStack

import concourse.bass as bass
import concourse.tile as tile
from concourse import bacc, mybir
from concourse.bass_utils import run_bass_kernel_spmd
from concourse.masks import make_identity

BF16 = mybir.dt.bfloat16
F32 = mybir.dt.float32
EXP = mybir.ActivationFunctionType.Exp
LN = mybir.ActivationFunctionType.Ln
NPBF16 = ml_dtypes.bfloat16

NCORES = 8
B, NSEQ, C, H, D = 2, 2048, 1024, 16, 64
T = B * NSEQ                 # 4096 flattened tokens
SCALE = D ** -0.5            # folded into the exp activation
NKC = C // 128               # 8 contraction chunks
IT = 512                     # query i-tile
NI = NSEQ // IT              # 4 i-tiles per batch
NJ = NSEQ // 128             # 16 key chunks per batch
JG = 2                       # key chunks per exp group ([128,1024] psum)
NG = NJ // JG                # 8 groups per (b, i)
STAGE_TOK = [1024, 1024, 1024, 1024]              # tokens per a2a stage
STAGE_OFF = [0, 1024, 2048, 3072]                 # global token offsets
TSL = T // NCORES            # 512 output tokens per core
DEBUG = False                # dump intermediates as extra outputs


def build_program():
    nc = bacc.Bacc("TRN2", target_bir_lowering=False, debug=False,
                   num_devices=NCORES)

    x_d = nc.dram_tensor("x", [T, C], BF16, kind="ExternalInput")
    wqk_d = nc.dram_tensor("wqk", [C, 256], BF16, kind="ExternalInput")
    wv_d = nc.dram_tensor("wv", [C, 128], BF16, kind="ExternalInput")
    wp_d = nc.dram_tensor("wproj", [C, C], BF16, kind="ExternalInput")
    bp_d = nc.dram_tensor("bproj", [1, C], BF16, kind="ExternalInput")
    y_d = nc.dram_tensor("y", [TSL, C], F32, kind="ExternalOutput")

    a2a_in = [nc.dram_tensor(f"a2a_in{q}", [NCORES * 128, STAGE_TOK[q] // NCORES],
                             BF16) for q in range(len(STAGE_TOK))]
    a2a_out = [nc.dram_tensor(f"a2a_out{q}", [NCORES * 128, STAGE_TOK[q] // NCORES],
                              BF16) for q in range(len(STAGE_TOK))]

    dbg = {}
    if DEBUG:
        dbg["qkv"] = nc.dram_tensor("dbg_qkv", [128, 3 * IT], BF16,
                                    kind="ExternalOutput")
        dbg["vni"] = nc.dram_tensor("dbg_vni", [128, 130], BF16,
                                    kind="ExternalOutput")
        dbg["ex"] = nc.dram_tensor("dbg_ex", [128, JG * IT], BF16,
                                   kind="ExternalOutput")
        dbg["rcp"] = nc.dram_tensor("dbg_rcp", [1, IT], F32,
                                    kind="ExternalOutput")
        dbg["outT"] = nc.dram_tensor("dbg_outT", [128, T], BF16,
                                     kind="ExternalOutput")
        dbg["bc"] = nc.dram_tensor("dbg_bc", [64, IT], BF16,
                                   kind="ExternalOutput")

    with tile.TileContext(nc) as tc, ExitStack() as ctx:
        ep = ctx.enter_context

        consts = ep(tc.tile_pool(name="consts", bufs=1))
        p_xt = ep(tc.tile_pool(name="xt", bufs=1))
        p_q = ep(tc.tile_pool(name="qz", bufs=1))
        p_k = ep(tc.tile_pool(name="kz", bufs=1))
        p_vt = ep(tc.tile_pool(name="vt", bufs=1))
        p_v = ep(tc.tile_pool(name="vni", bufs=1))
        p_exp = ep(tc.tile_pool(name="exps", bufs=4))
        p_outt = ep(tc.tile_pool(name="outt", bufs=1))
        p_nrm = ep(tc.tile_pool(name="nrm", bufs=2))
        p_ots = ep(tc.tile_pool(name="ots", bufs=2))
        p_y = ep(tc.tile_pool(name="ysb", bufs=2))
        ps_s = ep(tc.tile_pool(name="pss", bufs=2, space="PSUM"))
        ps_ou = ep(tc.tile_pool(name="psou", bufs=1, space="PSUM"))
        ps_mm = ep(tc.tile_pool(name="psmm", bufs=2, space="PSUM"))

        # ---- weights: one batched DMA per tensor ----
        wqk_sb = consts.tile([128, NKC, 256], BF16)
        wv_sb = consts.tile([128, NKC, 128], BF16)
        wp_sb = consts.tile([128, NKC, C], BF16)
        bp_sb = consts.tile([1, C], BF16)
        # Small weights first (wqk/wv needed by the first qkv matmuls), then
        # ALL transposes hoisted together (XBAR transpose mode is shared
        # state: never interleave copy-DMAs between transposes), then the
        # big wproj (2MB, needed only ~90us in).
        nc.sync.dma_start(out=wqk_sb[:],
                          in_=wqk_d.rearrange("(c p) j -> p c j", c=NKC))
        nc.sync.dma_start(out=wv_sb[:],
                          in_=wv_d.rearrange("(c p) j -> p c j", c=NKC))
        nc.sync.dma_start(out=bp_sb[:], in_=bp_d[0:1, :])

        ident = consts.tile([128, 128], BF16)
        make_identity(nc, ident[:])
        ones_row = consts.tile([1, 128], BF16)
        nc.vector.memset(ones_row[:], 1.0)

        # ---- x^T DMA transposes: per (b, tp, c) tile [128, 1024] ----
        xt = {}

        def issue_xt(b, tp, c, eng):
            xti = p_xt.tile([128, 1024], BF16, tag=f"xt{b}{tp}{c}",
                            name=f"xt{b}{tp}{c}")
            eng.dma_start_transpose(
                xti[:], x_d[b * NSEQ + tp * 1024: b * NSEQ + (tp + 1) * 1024,
                            c * 128:(c + 1) * 128])
            xt[(b, tp, c)] = xti

        for b in range(2):
            for tp in range(2):
                for c in range(NKC):
                    issue_xt(b, tp, c, nc.sync)

        nc.sync.dma_start(out=wp_sb[:],
                          in_=wp_d.rearrange("(c p) j -> p c j", c=NKC))

        # ---- cross-stage state ----
        qz, kz, vT = {}, {}, {}       # (b, tt) -> [128, 512] bf16
        vni = {}                      # (b, j) -> [128, 130] bf16
        outT = [p_outt.tile([64, T], BF16, tag=f"outT{h}", name=f"outT{h}")
                for h in range(2)]
        outu = {}                     # (b, i) -> [outu_A, outu_B] psum

        fillers = deque()

        def fill(n):
            for _ in range(min(n, len(fillers))):
                fillers.popleft()()

        # ---- qkv projection for one 512-token tile ----
        def qkv_tt(b, tt):
            t0 = tt * IT

            def emit(dst_pool, wsb, col0, tagn):
                ps = ps_mm.tile([128, IT], F32, tag="mm", name="ps")
                for c in range(NKC):
                    nc.tensor.matmul(
                        ps[:], wsb[:, c, col0:col0 + 128],
                        xt[(b, tt // 2, c)][:, (tt % 2) * IT:(tt % 2 + 1) * IT],
                        start=(c == 0), stop=(c == NKC - 1))
                dst = dst_pool.tile([128, IT], BF16, tag=f"{tagn}{b}{tt}",
                                    name=f"{tagn}{b}{tt}")
                nc.vector.tensor_copy(dst[:], ps[:])
                return dst

            qz[(b, tt)] = emit(p_q, wqk_sb, 0, "qz")
            kz[(b, tt)] = emit(p_k, wqk_sb, 128, "kz")
            vT[(b, tt)] = emit(p_vt, wv_sb, 0, "vT")

        # qkv as filler closures: 3 psum groups split into 2-matmul chunks
        def qkv_tt_closures(b, tt):
            out = []

            def make(dst_pool, wsb, col0, tagn):
                ps = ps_mm.tile([128, IT], F32, tag="mm", name="ps")

                def mk_mm(c0):
                    def f():
                        for c in (c0, c0 + 1):
                            nc.tensor.matmul(
                                ps[:], wsb[:, c, col0:col0 + 128],
                                xt[(b, tt // 2, c)][:, (tt % 2) * IT:
                                                    (tt % 2 + 1) * IT],
                                start=(c == 0), stop=(c == NKC - 1))
                    return f

                for c0 in range(0, NKC, 2):
                    out.append(mk_mm(c0))
                dst = dst_pool.tile([128, IT], BF16, tag=f"{tagn}{b}{tt}",
                                    name=f"{tagn}{b}{tt}")

                def cp():
                    nc.vector.tensor_copy(dst[:], ps[:])
                out.append(cp)
                return dst

            qz[(b, tt)] = make(p_q, wqk_sb, 0, "qz")
            kz[(b, tt)] = make(p_k, wqk_sb, 128, "kz")
            vT[(b, tt)] = make(p_vt, wv_sb, 0, "vT")
            return out

        # ---- v transpose + [v_A|1|v_B|1] staging for one key chunk ----
        def vn_j(b, j):
            vtr = ps_mm.tile([128, 128], BF16, tag="mm", name="vtr")
            nc.tensor.transpose(vtr[:],
                                vT[(b, j // 4)][:, (j % 4) * 128:(j % 4 + 1) * 128],
                                ident[:])
            vn = p_v.tile([128, 130], BF16, tag=f"v{b}{j}", name=f"v{b}{j}")
            nc.vector.memset(vn[:, 64:65], 1.0)
            nc.vector.memset(vn[:, 129:130], 1.0)
            nc.vector.tensor_copy(vn[:, 0:64], vtr[:, 0:64])
            nc.vector.tensor_copy(vn[:, 65:129], vtr[:, 64:128])
            vni[(b, j)] = vn

        # ---- normalization: 1/denom = exp(-ln(denom)) on ScalarE, then
        # partition-broadcast on the (idle) GpSimd engine, multiply on DVE.
        def norm(b, i):
            ou = outu.pop((b, i))
            t0 = b * NSEQ + i * IT
            for h in range(2):
                rf = p_nrm.tile([65, IT], F32, tag=f"rf{h}", name="rf")
                nc.scalar.activation(rf[64:65, :], ou[h][64:65, :], LN)
                rb = p_nrm.tile([65, IT], BF16, tag=f"rb{h}", name="rb")
                nc.scalar.activation(rb[64:65, :], rf[64:65, :], EXP,
                                     scale=-1.0)
                if DEBUG and (b, i, h) == (0, 0, 0):
                    nc.sync.dma_start(out=dbg["rcp"][0:1, :],
                                      in_=rf[64:65, :])
                # partition-move the reciprocal row to partition 0 via DMA
                # (engines are partition-wired; gpsimd broadcast reads p0)
                rb0 = p_nrm.tile([1, IT], BF16, tag=f"rb0{h}", name="rb0")
                nc.sync.dma_start(out=rb0[:], in_=rb[64:65, :])
                bc = p_nrm.tile([64, IT], BF16, tag=f"bc{h}", name="bc")
                nc.gpsimd.partition_broadcast(bc[:], rb0[:])
                if DEBUG and (b, i, h) == (0, 0, 0):
                    nc.sync.dma_start(out=dbg["bc"][:, :], in_=bc[:])
                nc.vector.tensor_mul(outT[h][:, t0:t0 + IT],
                                     ou[h][0:64, :], bc[:])

        # ---- attention for one (batch, i-tile), exp-paced with fillers ----
        def attn(b, i):
            ou = [ps_ou.tile([65, IT], F32, tag=f"ou{h}", name=f"ou{h}")
                  for h in range(2)]
            outu[(b, i)] = ou
            ex_prev = [None, None]
            ex_cur = [None, None]

            def s_group(h, g):
                st = ps_s.tile([128, JG * IT], F32, tag="s", name="s")
                for k in range(JG):
                    j = g * JG + k
                    nc.tensor.matmul(
                        st[:, k * IT:(k + 1) * IT],
                        kz[(b, j // 4)][h * 64:(h + 1) * 64,
                                        (j % 4) * 128:(j % 4 + 1) * 128],
                        qz[(b, i)][h * 64:(h + 1) * 64, :],
                        start=True, stop=True)
                ex = p_exp.tile([128, JG * IT], BF16, tag="ex", name="ex")
                nc.scalar.activation(ex[:], st[:], EXP, scale=SCALE)
                return ex

            def av(h, g, ex):
                for k in range(JG):
                    j = g * JG + k
                    nc.tensor.matmul(
                        ou[h][:], vni[(b, j)][:, h * 65:h * 65 + 65],
                        ex[:, k * IT:(k + 1) * IT],
                        start=(j == 0), stop=(j == NJ - 1))

            for g in range(NG):
                fill(1)
                ex_cur[0] = s_group(0, g)
                if DEBUG and (b, i, g) == (0, 0, 0):
                    nc.sync.dma_start(out=dbg["ex"][:, :], in_=ex_cur[0][:])
                if g > 0:
                    av(0, g - 1, ex_prev[0])
                fill(1)
                ex_cur[1] = s_group(1, g)
                if g > 0:
                    av(1, g - 1, ex_prev[1])
                fill(1)
                ex_prev = list(ex_cur)
            av(0, NG - 1, ex_prev[0])
            av(1, NG - 1, ex_prev[1])

        # ---- a2a staging + collective for one stage ----
        def stage_a2a(q):
            tok, off = STAGE_TOK[q], STAGE_OFF[q]
            tf = tok // NCORES
            for h in range(2):
                nc.sync.dma_start(
                    out=a2a_in[q].rearrange("(s x) c -> x s c",
                                            s=NCORES)[h * 64:(h + 1) * 64],
                    in_=outT[h][:, off:off + tok].rearrange(
                        "r (s c) -> r s c", s=NCORES))
            nc.gpsimd.collective_compute(
                "AllToAll", mybir.AluOpType.bypass,
                replica_groups=[list(range(NCORES))],
                ins=[a2a_in[q][:, :]], outs=[a2a_out[q][:, :]])

        # ---- output projection for one stage ----
        def oproj_closures(q):
            tok = STAGE_TOK[q]
            tf = tok // NCORES
            yoff = sum(STAGE_TOK[:q]) // NCORES
            out = []
            ots = p_ots.tile([128, NCORES, tf], BF16, tag="ots", name="ots")

            def unstage():
                nc.sync.dma_start(
                    out=ots[:],
                    in_=a2a_out[q].rearrange("(s p) c -> p s c", s=NCORES))
            out.append(unstage)
            y_ps = [ps_mm.tile([tf, IT], F32, tag="mm", name="yps")
                    for _ in range(2)]

            def mk_mm(s):
                def f():
                    for n in range(2):
                        nc.tensor.matmul(
                            y_ps[n][:], ots[:, s, :],
                            wp_sb[:, s, n * IT:(n + 1) * IT],
                            start=(s == 0), stop=False)
                return f

            for s in range(NKC):
                out.append(mk_mm(s))

            def bias_and_out():
                y_sb = p_y.tile([tf, C], F32, tag="y", name="ysb")
                for n in range(2):
                    nc.tensor.matmul(y_ps[n][:], ones_row[:, 0:tf],
                                     bp_sb[:, n * IT:(n + 1) * IT],
                                     start=False, stop=True)
                for n in range(2):
                    nc.vector.tensor_copy(y_sb[:, n * IT:(n + 1) * IT],
                                          y_ps[n][:])
                nc.sync.dma_start(out=y_d[yoff:yoff + tf, :], in_=y_sb[:])
            out.append(bias_and_out)
            return out

        # ================= emission schedule =================
        for tt in range(4):
            qkv_tt(0, tt)
        for j in range(NJ):
            vn_j(0, j)

        if DEBUG:
            nc.sync.dma_start(out=dbg["qkv"][:, 0:IT], in_=qz[(0, 0)][:])
            nc.sync.dma_start(out=dbg["qkv"][:, IT:2 * IT], in_=kz[(0, 0)][:])
            nc.sync.dma_start(out=dbg["qkv"][:, 2 * IT:3 * IT], in_=vT[(0, 0)][:])
            nc.sync.dma_start(out=dbg["vni"][:, :], in_=vni[(0, 0)][:])

        fillers.extend(qkv_tt_closures(1, 0))
        attn(0, 0)
        norm(0, 0)

        fillers.extend(qkv_tt_closures(1, 1))
        attn(0, 1)
        norm(0, 1)
        stage_a2a(0)

        fillers.extend(qkv_tt_closures(1, 2))
        fillers.extend([(lambda j=j: vn_j(1, j)) for j in range(NJ // 2)])
        attn(0, 2)
        norm(0, 2)

        fillers.extend(qkv_tt_closures(1, 3))
        fillers.extend([(lambda j=j: vn_j(1, j)) for j in range(NJ // 2, NJ)])
        attn(0, 3)
        norm(0, 3)
        stage_a2a(1)

        attn(1, 0)
        norm(1, 0)

        fillers.extend(oproj_closures(0))
        attn(1, 1)
        norm(1, 1)
        stage_a2a(2)

        fillers.extend(oproj_closures(1))
        attn(1, 2)
        norm(1, 2)

        attn(1, 3)
        norm(1, 3)
        stage_a2a(3)

        # oproj(2) runs post-attention, covering the exposed last collective
        for f in oproj_closures(2):
            f()
        fill(len(fillers))
        for f in oproj_closures(3):
            f()

        if DEBUG:
            nc.sync.dma_start(out=dbg["outT"][0:64, :], in_=outT[0][:])
            nc.sync.dma_start(out=dbg["outT"][64:128, :], in_=outT[1][:])

    nc.compile()
    return nc


_NC = None


def _get_nc():
    global _NC
    if _NC is None:
        _NC = build_program()
    return _NC


def prep_in_maps(x, w_qkv, w_proj, b_proj):
    x_bf = np.ascontiguousarray(np.asarray(x, dtype=np.float32).reshape(T, C)
                                ).astype(NPBF16)
    w_qkv = np.asarray(w_qkv, dtype=np.float32)
    w_proj = np.asarray(w_proj, dtype=np.float32)
    b_proj = np.asarray(b_proj, dtype=np.float32)
    wp_bf = np.ascontiguousarray(w_proj).astype(NPBF16)
    bp_bf = b_proj.reshape(1, C).astype(NPBF16)

    q_w, k_w, v_w = w_qkv[:, 0:C], w_qkv[:, C:2 * C], w_qkv[:, 2 * C:3 * C]
    in_maps = []
    for c in range(NCORES):
        hA, hB = 2 * c, 2 * c + 1
        sA, sB = slice(hA * D, (hA + 1) * D), slice(hB * D, (hB + 1) * D)
        wqk_c = np.concatenate([q_w[:, sA], q_w[:, sB], k_w[:, sA], k_w[:, sB]],
                               axis=1).astype(NPBF16)
        wv_c = np.concatenate([v_w[:, sA], v_w[:, sB]], axis=1).astype(NPBF16)
        in_maps.append({"x": x_bf, "wqk": np.ascontiguousarray(wqk_c),
                        "wv": np.ascontiguousarray(wv_c), "wproj": wp_bf,
                        "bproj": bp_bf})
    return in_maps


def assemble(results):
    y = np.empty((T, C), dtype=np.float32)
    for c in range(NCORES):
        yc = results[c]["y"]
        for q, (tok, off) in enumerate(zip(STAGE_TOK, STAGE_OFF)):
            tf = tok // NCORES
            yoff = sum(STAGE_TOK[:q]) // NCORES
            g0 = off + c * tf
            y[g0:g0 + tf, :] = yc[yoff:yoff + tf, :]
    return y.reshape(B, NSEQ, C)


def run(in_maps, trace=False):
    nc = _get_nc()
    return run_bass_kernel_spmd(nc, in_maps, core_ids=list(range(NCORES)),
                                trace=trace)


def kernel(x, w_qkv, w_proj, b_proj):
    res = run(prep_in_maps(x, w_qkv, w_proj, b_proj))
    return assemble(res.results)


# revision 28
# speedup vs baseline: 1.1018x; 1.1018x over previous
"""Multi-head attention (B=2, N=2048, C=1024, H=16) on 8 Trainium2 NeuronCores.

Sharding: tensor-parallel over heads (2 heads/core) for qkv-proj + attention;
all-to-all of the attention output in 5 stages (1024,1024,1024,512,512 tokens,
smaller tail stages to shrink the serial end), then each core runs the output
projection over the full channel dim for its token slices.

Per-core layouts:
  x^T   32 tiles [128, 1024] via HW DMA-transpose, issued on BOTH hwdge
        queues (sync + scalar) so the first qkv matmul starts ~5us in
  q^T/k^T/v^T per 512-token tile [128, 512]: rows 0:63 head A, 64:127 head B
        (natural packing; S matmuls use K=64 with tile_position from base
        partition -- no zero-padding, one PSUM->SBUF copy per tile)
  S^T   [128 keys, 1024] psum groups (2 key-chunks x 512 queries), 2 bufs
        cycling heads; exp on ScalarE (bf16 out, scale folded)
  vni   [128, 130] = [v_A|1|v_B|1] per key-chunk (ones col -> softmax denom)
  out_u [65, 512] psum accumulated over all 16 key chunks; normalization
        reads psum directly: reciprocal_approx_fast + ones-row broadcast
        matmul + DVE mul -> outT[h] [64, 4096] bf16
  a2a   one staging DMA per (stage, head) via rearranged APs; one unstage
        DMA per stage; oproj accumulates 8 K-chunks + bias matmul

Emission order software-pipelines the engines: the exp-bound attention inner
loop is padded with "filler" closures (qkv of next batch, v transposes,
oproj of previous stage) popped between groups so the PE never idles (keeps
the tensor-engine p-state at full clock).
"""

import numpy as np
import ml_dtypes
from collections import deque
from contextlib import ExitStack

import concourse.bass as bass
import concourse.tile as tile
from concourse import bacc, mybir
from concourse.bass_utils import run_bass_kernel_spmd
from concourse.masks import make_identity

BF16 = mybir.dt.bfloat16
F32 = mybir.dt.float32
EXP = mybir.ActivationFunctionType.Exp
LN = mybir.ActivationFunctionType.Ln
NPBF16 = ml_dtypes.bfloat16

NCORES = 8
B, NSEQ, C, H, D = 2, 2048, 1024, 16, 64
T = B * NSEQ                 # 4096 flattened tokens
SCALE = D ** -0.5            # folded into the exp activation
NKC = C // 128               # 8 contraction chunks
IT = 512                     # query i-tile
NI = NSEQ // IT              # 4 i-tiles per batch
NJ = NSEQ // 128             # 16 key chunks per batch
JG = 2                       # key chunks per exp group ([128,1024] psum)
NG = NJ // JG                # 8 groups per (b, i)
STAGE_TOK = [1024, 1024, 1024, 1024]              # tokens per a2a stage
STAGE_OFF = [0, 1024, 2048, 3072]                 # global token offsets
TSL = T // NCORES            # 512 output tokens per core
DEBUG = False                # dump intermediates as extra outputs


def build_program():
    nc = bacc.Bacc("TRN2", target_bir_lowering=False, debug=False,
                   num_devices=NCORES)

    x_d = nc.dram_tensor("x", [T, C], BF16, kind="ExternalInput")
    wqk_d = nc.dram_tensor("wqk", [C, 256], BF16, kind="ExternalInput")
    wv_d = nc.dram_tensor("wv", [C, 128], BF16, kind="ExternalInput")
    wp_d = nc.dram_tensor("wproj", [C, C], BF16, kind="ExternalInput")
    bp_d = nc.dram_tensor("bproj", [1, C], BF16, kind="ExternalInput")
    y_d = nc.dram_tensor("y", [TSL, C], F32, kind="ExternalOutput")

    a2a_in = [nc.dram_tensor(f"a2a_in{q}", [NCORES * 128, STAGE_TOK[q] // NCORES],
                             BF16) for q in range(len(STAGE_TOK))]
    a2a_out = [nc.dram_tensor(f"a2a_out{q}", [NCORES * 128, STAGE_TOK[q] // NCORES],
                              BF16) for q in range(len(STAGE_TOK))]

    dbg = {}
    if DEBUG:
        dbg["qkv"] = nc.dram_tensor("dbg_qkv", [128, 3 * IT], BF16,
                                    kind="ExternalOutput")
        dbg["vni"] = nc.dram_tensor("dbg_vni", [128, 130], BF16,
                                    kind="ExternalOutput")
        dbg["ex"] = nc.dram_tensor("dbg_ex", [128, JG * IT], BF16,
                                   kind="ExternalOutput")
        dbg["rcp"] = nc.dram_tensor("dbg_rcp", [1, IT], F32,
                                    kind="ExternalOutput")
        dbg["outT"] = nc.dram_tensor("dbg_outT", [128, T], BF16,
                                     kind="ExternalOutput")
        dbg["bc"] = nc.dram_tensor("dbg_bc", [64, IT], BF16,
                                   kind="ExternalOutput")

    with tile.TileContext(nc) as tc, ExitStack() as ctx:
        ep = ctx.enter_context

        consts = ep(tc.tile_pool(name="consts", bufs=1))
        p_xt = ep(tc.tile_pool(name="xt", bufs=1))
        p_q = ep(tc.tile_pool(name="qz", bufs=1))
        p_k = ep(tc.tile_pool(name="kz", bufs=1))
        p_vt = ep(tc.tile_pool(name="vt", bufs=1))
        p_v = ep(tc.tile_pool(name="vni", bufs=1))
        p_exp = ep(tc.tile_pool(name="exps", bufs=4))
        p_outt = ep(tc.tile_pool(name="outt", bufs=1))
        p_nrm = ep(tc.tile_pool(name="nrm", bufs=2))
        p_ots = ep(tc.tile_pool(name="ots", bufs=2))
        p_y = ep(tc.tile_pool(name="ysb", bufs=2))
        ps_s = ep(tc.tile_pool(name="pss", bufs=2, space="PSUM"))
        ps_ou = ep(tc.tile_pool(name="psou", bufs=1, space="PSUM"))
        ps_mm = ep(tc.tile_pool(name="psmm", bufs=2, space="PSUM"))

        # ---- weights: one batched DMA per tensor ----
        wqk_sb = consts.tile([128, NKC, 256], BF16)
        wv_sb = consts.tile([128, NKC, 128], BF16)
        wp_sb = consts.tile([128, NKC, C], BF16)
        bp_sb = consts.tile([1, C], BF16)
        # Small weights first (wqk/wv needed by the first qkv matmuls), then
        # ALL transposes hoisted together (XBAR transpose mode is shared
        # state: never interleave copy-DMAs between transposes), then the
        # big wproj (2MB, needed only ~90us in).
        nc.sync.dma_start(out=wqk_sb[:],
                          in_=wqk_d.rearrange("(c p) j -> p c j", c=NKC))
        nc.sync.dma_start(out=wv_sb[:],
                          in_=wv_d.rearrange("(c p) j -> p c j", c=NKC))
        nc.sync.dma_start(out=bp_sb[:], in_=bp_d[0:1, :])

        ident = consts.tile([128, 128], BF16)
        make_identity(nc, ident[:])
        ones_row = consts.tile([1, 128], BF16)
        nc.vector.memset(ones_row[:], 1.0)

        # ---- x^T DMA transposes: per (b, tp, c) tile [128, 1024] ----
        xt = {}

        def issue_xt(b, tp, c, eng):
            xti = p_xt.tile([128, 1024], BF16, tag=f"xt{b}{tp}{c}",
                            name=f"xt{b}{tp}{c}")
            eng.dma_start_transpose(
                xti[:], x_d[b * NSEQ + tp * 1024: b * NSEQ + (tp + 1) * 1024,
                            c * 128:(c + 1) * 128])
            xt[(b, tp, c)] = xti

        for b in range(2):
            for tp in range(2):
                for c in range(NKC):
                    issue_xt(b, tp, c, nc.sync)

        nc.sync.dma_start(out=wp_sb[:],
                          in_=wp_d.rearrange("(c p) j -> p c j", c=NKC))

        # ---- cross-stage state ----
        qz, kz, vT = {}, {}, {}       # (b, tt) -> [128, 512] bf16
        vni = {}                      # (b, j) -> [128, 130] bf16
        outT = [p_outt.tile([64, T], BF16, tag=f"outT{h}", name=f"outT{h}")
                for h in range(2)]
        outu = {}                     # (b, i) -> [outu_A, outu_B] psum

        fillers = deque()

        def fill(n):
            for _ in range(min(n, len(fillers))):
                fillers.popleft()()

        # ---- qkv projection for one 512-token tile ----
        def qkv_tt(b, tt):
            t0 = tt * IT

            def emit(dst_pool, wsb, col0, tagn):
                ps = ps_mm.tile([128, IT], F32, tag="mm", name="ps")
                for c in range(NKC):
                    nc.tensor.matmul(
                        ps[:], wsb[:, c, col0:col0 + 128],
                        xt[(b, tt // 2, c)][:, (tt % 2) * IT:(tt % 2 + 1) * IT],
                        start=(c == 0), stop=(c == NKC - 1))
                dst = dst_pool.tile([128, IT], BF16, tag=f"{tagn}{b}{tt}",
                                    name=f"{tagn}{b}{tt}")
                nc.vector.tensor_copy(dst[:], ps[:])
                return dst

            qz[(b, tt)] = emit(p_q, wqk_sb, 0, "qz")
            kz[(b, tt)] = emit(p_k, wqk_sb, 128, "kz")
            vT[(b, tt)] = emit(p_vt, wv_sb, 0, "vT")

        # qkv as filler closures: 3 psum groups split into 2-matmul chunks
        def qkv_tt_closures(b, tt):
            out = []

            def make(dst_pool, wsb, col0, tagn):
                ps = ps_mm.tile([128, IT], F32, tag="mm", name="ps")

                def mk_mm(c0):
                    def f():
                        for c in (c0, c0 + 1):
                            nc.tensor.matmul(
                                ps[:], wsb[:, c, col0:col0 + 128],
                                xt[(b, tt // 2, c)][:, (tt % 2) * IT:
                                                    (tt % 2 + 1) * IT],
                                start=(c == 0), stop=(c == NKC - 1))
                    return f

                for c0 in range(0, NKC, 2):
                    out.append(mk_mm(c0))
                dst = dst_pool.tile([128, IT], BF16, tag=f"{tagn}{b}{tt}",
                                    name=f"{tagn}{b}{tt}")

                def cp():
                    nc.vector.tensor_copy(dst[:], ps[:])
                out.append(cp)
                return dst

            qz[(b, tt)] = make(p_q, wqk_sb, 0, "qz")
            kz[(b, tt)] = make(p_k, wqk_sb, 128, "kz")
            vT[(b, tt)] = make(p_vt, wv_sb, 0, "vT")
            return out

        # ---- v transpose + [v_A|1|v_B|1] staging for one key chunk ----
        def vn_j(b, j):
            vtr = ps_mm.tile([128, 128], BF16, tag="mm", name="vtr")
            nc.tensor.transpose(vtr[:],
                                vT[(b, j // 4)][:, (j % 4) * 128:(j % 4 + 1) * 128],
                                ident[:])
            vn = p_v.tile([128, 130], BF16, tag=f"v{b}{j}", name=f"v{b}{j}")
            nc.vector.memset(vn[:, 64:65], 1.0)
            nc.vector.memset(vn[:, 129:130], 1.0)
            nc.vector.tensor_copy(vn[:, 0:64], vtr[:, 0:64])
            nc.vector.tensor_copy(vn[:, 65:129], vtr[:, 64:128])
            vni[(b, j)] = vn

        # ---- normalization ----
        # Stage outu to SBUF first (frees the PSUM accumulator immediately so
        # the next i-tile's AV never waits on this chain), then reciprocal on
        # DVE, partition-move the row to p0 via DMA (engines are partition-
        # wired), broadcast on the idle GpSimd engine, multiply on DVE.
        def norm(b, i):
            ou = outu.pop((b, i))
            t0 = b * NSEQ + i * IT
            for h in range(2):
                ouc = p_nrm.tile([65, IT], F32, tag=f"ouc{h}", name="ouc")
                nc.vector.tensor_copy(ouc[:], ou[h][0:65, :])
                rf = p_nrm.tile([65, IT], F32, tag=f"rf{h}", name="rf")
                nc.vector.reciprocal(rf[64:65, :], ouc[64:65, :])
                rb = p_nrm.tile([65, IT], BF16, tag=f"rb{h}", name="rb")
                nc.vector.tensor_copy(rb[64:65, :], rf[64:65, :])
                if DEBUG and (b, i, h) == (0, 0, 0):
                    nc.sync.dma_start(out=dbg["rcp"][0:1, :],
                                      in_=rf[64:65, :])
                rb0 = p_nrm.tile([1, IT], BF16, tag=f"rb0{h}", name="rb0")
                nc.sync.dma_start(out=rb0[:], in_=rb[64:65, :])
                bc = p_nrm.tile([64, IT], BF16, tag=f"bc{h}", name="bc")
                nc.gpsimd.partition_broadcast(bc[:], rb0[:])
                if DEBUG and (b, i, h) == (0, 0, 0):
                    nc.sync.dma_start(out=dbg["bc"][:, :], in_=bc[:])
                nc.vector.tensor_mul(outT[h][:, t0:t0 + IT],
                                     ouc[0:64, :], bc[:])

        # ---- attention for one (batch, i-tile), exp-paced with fillers ----
        def attn(b, i):
            ou = [ps_ou.tile([65, IT], F32, tag=f"ou{h}", name=f"ou{h}")
                  for h in range(2)]
            outu[(b, i)] = ou
            ex_prev = [None, None]
            ex_cur = [None, None]

            def s_group(h, g):
                st = ps_s.tile([128, JG * IT], F32, tag="s", name="s")
                for k in range(JG):
                    j = g * JG + k
                    nc.tensor.matmul(
                        st[:, k * IT:(k + 1) * IT],
                        kz[(b, j // 4)][h * 64:(h + 1) * 64,
                                        (j % 4) * 128:(j % 4 + 1) * 128],
                        qz[(b, i)][h * 64:(h + 1) * 64, :],
                        start=True, stop=True)
                ex = p_exp.tile([128, JG * IT], BF16, tag="ex", name="ex")
                nc.scalar.activation(ex[:], st[:], EXP, scale=SCALE)
                return ex

            def av(h, g, ex):
                for k in range(JG):
                    j = g * JG + k
                    nc.tensor.matmul(
                        ou[h][:], vni[(b, j)][:, h * 65:h * 65 + 65],
                        ex[:, k * IT:(k + 1) * IT],
                        start=(j == 0), stop=(j == NJ - 1))

            for g in range(NG):
                fill(1)
                ex_cur[0] = s_group(0, g)
                if DEBUG and (b, i, g) == (0, 0, 0):
                    nc.sync.dma_start(out=dbg["ex"][:, :], in_=ex_cur[0][:])
                if g > 0:
                    av(0, g - 1, ex_prev[0])
                fill(1)
                ex_cur[1] = s_group(1, g)
                if g > 0:
                    av(1, g - 1, ex_prev[1])
                fill(1)
                ex_prev = list(ex_cur)
            av(0, NG - 1, ex_prev[0])
            av(1, NG - 1, ex_prev[1])

        # ---- a2a staging + collective for one stage ----
        def stage_a2a(q):
            tok, off = STAGE_TOK[q], STAGE_OFF[q]
            tf = tok // NCORES
            for h in range(2):
                nc.sync.dma_start(
                    out=a2a_in[q].rearrange("(s x) c -> x s c",
                                            s=NCORES)[h * 64:(h + 1) * 64],
                    in_=outT[h][:, off:off + tok].rearrange(
                        "r (s c) -> r s c", s=NCORES))
            nc.gpsimd.collective_compute(
                "AllToAll", mybir.AluOpType.bypass,
                replica_groups=[list(range(NCORES))],
                ins=[a2a_in[q][:, :]], outs=[a2a_out[q][:, :]])

        # ---- output projection for one stage ----
        def oproj_closures(q):
            tok = STAGE_TOK[q]
            tf = tok // NCORES
            yoff = sum(STAGE_TOK[:q]) // NCORES
            out = []
            ots = p_ots.tile([128, NCORES, tf], BF16, tag="ots", name="ots")

            def unstage():
                nc.sync.dma_start(
                    out=ots[:],
                    in_=a2a_out[q].rearrange("(s p) c -> p s c", s=NCORES))
            out.append(unstage)
            y_ps = [ps_mm.tile([tf, IT], F32, tag="mm", name="yps")
                    for _ in range(2)]

            def mk_mm(s):
                def f():
                    for n in range(2):
                        nc.tensor.matmul(
                            y_ps[n][:], ots[:, s, :],
                            wp_sb[:, s, n * IT:(n + 1) * IT],
                            start=(s == 0), stop=False)
                return f

            for s in range(NKC):
                out.append(mk_mm(s))

            def bias_and_out():
                y_sb = p_y.tile([tf, C], F32, tag="y", name="ysb")
                for n in range(2):
                    nc.tensor.matmul(y_ps[n][:], ones_row[:, 0:tf],
                                     bp_sb[:, n * IT:(n + 1) * IT],
                                     start=False, stop=True)
                for n in range(2):
                    nc.vector.tensor_copy(y_sb[:, n * IT:(n + 1) * IT],
                                          y_ps[n][:])
                nc.sync.dma_start(out=y_d[yoff:yoff + tf, :], in_=y_sb[:])
            out.append(bias_and_out)
            return out

        # ================= emission schedule =================
        for tt in range(4):
            qkv_tt(0, tt)
        for j in range(NJ):
            vn_j(0, j)

        if DEBUG:
            nc.sync.dma_start(out=dbg["qkv"][:, 0:IT], in_=qz[(0, 0)][:])
            nc.sync.dma_start(out=dbg["qkv"][:, IT:2 * IT], in_=kz[(0, 0)][:])
            nc.sync.dma_start(out=dbg["qkv"][:, 2 * IT:3 * IT], in_=vT[(0, 0)][:])
            nc.sync.dma_start(out=dbg["vni"][:, :], in_=vni[(0, 0)][:])

        fillers.extend(qkv_tt_closures(1, 0))
        attn(0, 0)
        norm(0, 0)

        fillers.extend(qkv_tt_closures(1, 1))
        attn(0, 1)
        norm(0, 1)
        stage_a2a(0)

        fillers.extend(qkv_tt_closures(1, 2))
        fillers.extend([(lambda j=j: vn_j(1, j)) for j in range(NJ // 2)])
        attn(0, 2)
        norm(0, 2)

        fillers.extend(qkv_tt_closures(1, 3))
        fillers.extend([(lambda j=j: vn_j(1, j)) for j in range(NJ // 2, NJ)])
        attn(0, 3)
        norm(0, 3)
        stage_a2a(1)

        attn(1, 0)
        norm(1, 0)

        fillers.extend(oproj_closures(0))
        attn(1, 1)
        norm(1, 1)
        stage_a2a(2)

        fillers.extend(oproj_closures(1))
        attn(1, 2)
        norm(1, 2)

        attn(1, 3)
        norm(1, 3)
        # oproj(2) emitted BEFORE stage_a2a(3): its unstage DMA must not
        # queue behind the tail staging on sync, and its matmuls cover the
        # exposed last collective.
        oproj2 = oproj_closures(2)
        oproj2[0]()                 # unstage DMA (a2a(2) completed long ago)
        stage_a2a(3)
        for f in oproj2[1:]:
            f()
        fill(len(fillers))
        for f in oproj_closures(3):
            f()

        if DEBUG:
            nc.sync.dma_start(out=dbg["outT"][0:64, :], in_=outT[0][:])
            nc.sync.dma_start(out=dbg["outT"][64:128, :], in_=outT[1][:])

    nc.compile()
    return nc


_NC = None


def _get_nc():
    global _NC
    if _NC is None:
        _NC = build_program()
    return _NC


def prep_in_maps(x, w_qkv, w_proj, b_proj):
    x_bf = np.ascontiguousarray(np.asarray(x, dtype=np.float32).reshape(T, C)
                                ).astype(NPBF16)
    w_qkv = np.asarray(w_qkv, dtype=np.float32)
    w_proj = np.asarray(w_proj, dtype=np.float32)
    b_proj = np.asarray(b_proj, dtype=np.float32)
    wp_bf = np.ascontiguousarray(w_proj).astype(NPBF16)
    bp_bf = b_proj.reshape(1, C).astype(NPBF16)

    q_w, k_w, v_w = w_qkv[:, 0:C], w_qkv[:, C:2 * C], w_qkv[:, 2 * C:3 * C]
    in_maps = []
    for c in range(NCORES):
        hA, hB = 2 * c, 2 * c + 1
        sA, sB = slice(hA * D, (hA + 1) * D), slice(hB * D, (hB + 1) * D)
        wqk_c = np.concatenate([q_w[:, sA], q_w[:, sB], k_w[:, sA], k_w[:, sB]],
                               axis=1).astype(NPBF16)
        wv_c = np.concatenate([v_w[:, sA], v_w[:, sB]], axis=1).astype(NPBF16)
        in_maps.append({"x": x_bf, "wqk": np.ascontiguousarray(wqk_c),
                        "wv": np.ascontiguousarray(wv_c), "wproj": wp_bf,
                        "bproj": bp_bf})
    return in_maps


def assemble(results):
    y = np.empty((T, C), dtype=np.float32)
    for c in range(NCORES):
        yc = results[c]["y"]
        for q, (tok, off) in enumerate(zip(STAGE_TOK, STAGE_OFF)):
            tf = tok // NCORES
            yoff = sum(STAGE_TOK[:q]) // NCORES
            g0 = off + c * tf
            y[g0:g0 + tf, :] = yc[yoff:yoff + tf, :]
    return y.reshape(B, NSEQ, C)


def run(in_maps, trace=False):
    nc = _get_nc()
    return run_bass_kernel_spmd(nc, in_maps, core_ids=list(range(NCORES)),
                                trace=trace)


def kernel(x, w_qkv, w_proj, b_proj):
    res = run(prep_in_maps(x, w_qkv, w_proj, b_proj))
    return assemble(res.results)
